# revision 18
# baseline (speedup 1.0000x reference)
"""CLS-AttentionPool2d Trainium2 kernel (8 NeuronCores, data-parallel over batch).

v4: host folds pos_emb into x (both layouts, bf16) so scores/weighted-sums
need no separate kp/postok tensors; all stationaries bf16 (FWL). DMA order
params -> xc g0 -> xc g1 -> xt g0 -> params2 -> xt g1 on the sync HWDGE ring
so both groups' score chains run during the xt stream; program phases
interleaved A0 B0 A1 B1 C0 C1. PE warmup burst beats the HAM cold clock.

Math (single CLS query => tiny attention):
  xp        = x + pos[1:]^T (host)
  mean      = xp.mean(j)  (= mean_x + posmean)
  q  = ISQ*Wq@mean + qc,  qc = ISQ*(Wq@pos0adj + bq),  pos0adj = pos0-posmean
  qblk[k, u] = q[k] * [head(k) == u]
  m  = Wk.T @ qblk
  scores[(s,h), j] = m.T @ xp ;  scores[., cls] = rowmean(scores) + m.T@pos0adj
  p = softmax(scores);  p' = p_tok + p_cls/1024
  w  = (p'.T @ xp^T + p_cls * pos0adj) / Z
  out = Wv @ w + bv   (per-head block of Wv)
"""

import math
import numpy as np

import concourse.bass as bass
import concourse.mybir as mybir
import concourse.tile as tile
from concourse import bacc
from concourse.bass import ts
from concourse.bass_utils import run_bass_kernel_spmd

F32 = mybir.dt.float32
BF16 = mybir.dt.bfloat16
AX = mybir.AxisListType
ALU = mybir.AluOpType
ACTF = mybir.ActivationFunctionType

B, C, HW = 64, 512, 1024
NH, DH = 8, 64
NCORES = 8
BPC = B // NCORES          # 8 batches per core
GRP = 4                    # batches per group (2 groups per core)
NGRP = BPC // GRP
CT = C // 128              # 4 c-chunks
JT = HW // 128             # 8 j-chunks
ISQ = 1.0 / math.sqrt(C)

_CACHE = {}


def _build_nc():
    nc = bacc.Bacc("TRN2", target_bir_lowering=False, debug=False,
                   num_devices=NCORES)

    # ---- DRAM I/O ----
    xc_d = nc.dram_tensor("xc", [BPC, 128, CT, HW], BF16, kind="ExternalInput")
    xt_d = nc.dram_tensor("xt", [BPC, 128, JT, C], BF16, kind="ExternalInput")
    wqt = nc.dram_tensor("wqt", [128, CT, C], BF16, kind="ExternalInput")
    wk = nc.dram_tensor("wk", [128, CT, C], BF16, kind="ExternalInput")
    wvt = nc.dram_tensor("wvt", [128, CT, C], BF16, kind="ExternalInput")
    pos0col = nc.dram_tensor("pos0col", [128, CT], BF16, kind="ExternalInput")
    pos0bc = nc.dram_tensor("pos0bc", [128, C], BF16, kind="ExternalInput")
    qc = nc.dram_tensor("qc", [128, CT], F32, kind="ExternalInput")
    bv = nc.dram_tensor("bv", [128, CT], F32, kind="ExternalInput")
    mask32 = nc.dram_tensor("mask32", [128, CT, 32], BF16, kind="ExternalInput")
    ident = nc.dram_tensor("ident", [128, 128], BF16, kind="ExternalInput")
    ident32 = nc.dram_tensor("ident32", [128, 32], BF16, kind="ExternalInput")
    out_d = nc.dram_tensor("out", [128, NGRP, CT, GRP], F32,
                           kind="ExternalOutput")

    with tile.TileContext(nc) as tc:
        with (
            tc.tile_pool(name="persist", bufs=1) as pp,
            tc.tile_pool(name="work", bufs=2) as wp,
            tc.tile_pool(name="psSC", bufs=1, space="PSUM") as psSC,
            tc.tile_pool(name="psW", bufs=1, space="PSUM") as psW,
            tc.tile_pool(name="psT", bufs=1, space="PSUM") as psT,
            tc.tile_pool(name="psQ", bufs=2, space="PSUM") as psQ,
        ):
            # ---- persistent tiles ----
            ident_s = pp.tile([128, 128], BF16)
            id32_s = pp.tile([128, 32], BF16)
            wk_s = pp.tile([128, CT, C], BF16)
            wqt_s = pp.tile([128, CT, C], BF16)
            qc_s = pp.tile([128, CT], F32)
            mask_s = pp.tile([128, CT, 32], BF16)
            pos0col_s = pp.tile([128, CT], BF16)
            pos0bc_s = pp.tile([128, C], BF16)
            wvt_s = pp.tile([128, CT, C], BF16)
            bv_s = pp.tile([128, CT], F32)
            xcs = [[pp.tile([128, 2, HW], BF16, name=f"xc{i}h{h}")
                    for h in range(2)] for i in range(BPC)]
            xts = [[pp.tile([128, 4, C], BF16, name=f"xt{i}h{h}")
                    for h in range(2)] for i in range(BPC)]
            junk = pp.tile([128, HW], BF16)

            # ---- ALL load DMAs up-front on the sync HWDGE ring (FIFO):
            # params -> xc g0 -> xc g1 -> xt g0 -> params2 -> xt g1
            for dst, src_ in [(ident_s, ident), (id32_s, ident32),
                              (qc_s, qc), (mask_s, mask32),
                              (pos0col_s, pos0col), (pos0bc_s, pos0bc)]:
                nc.sync.dma_start(out=dst[:], in_=src_[:])
            for h in range(2):
                nc.sync.dma_start(out=xcs[0][h][:],
                                  in_=xc_d[0][:, 2 * h:2 * h + 2, :])
            nc.sync.dma_start(out=wk_s[:], in_=wk[:])
            nc.sync.dma_start(out=wqt_s[:], in_=wqt[:])
            for s in range(1, GRP):
                for h in range(2):
                    nc.sync.dma_start(out=xcs[s][h][:],
                                      in_=xc_d[s][:, 2 * h:2 * h + 2, :])
            for s in range(GRP, BPC):
                for h in range(2):
                    nc.sync.dma_start(out=xts[s][h][:],
                                      in_=xt_d[s][:, 4 * h:4 * h + 4, :])
            nc.sync.dma_start(out=wvt_s[:], in_=wvt[:])
            nc.sync.dma_start(out=bv_s[:], in_=bv[:])
            for s in range(GRP, BPC):
                for h in range(2):
                    nc.sync.dma_start(out=xcs[s][h][:],
                                      in_=xc_d[s][:, 2 * h:2 * h + 2, :])
            for s in range(GRP):
                for h in range(2):
                    nc.sync.dma_start(out=xts[s][h][:],
                                      in_=xt_d[s][:, 4 * h:4 * h + 4, :])

            # ---- PE warmup burst (HAM unthrottle) while DMAs stream ----
            warm_ps = psT.tile([128, JT, 128], BF16, tag="tp")
            for _ in range(24):
                nc.tensor.transpose(warm_ps[:, 0, :], ident_s[:], ident_s[:])
            for s, h in ((0, 0), (0, 1), (1, 0), (1, 1)):
                for _ in range(3):
                    nc.tensor.transpose(warm_ps[:, 0, :],
                                        xcs[s][h][:, 0, 0:128], ident_s[:])

            # ---------------- phase bodies ----------------
            def phase_a(g):
                """Per-slot: mean -> q -> qblk -> m -> token+cls scores."""
                sc_ps = psSC.tile([128, 3, 512], F32, tag="sc")
                for s in range(GRP):
                    xca, xcb = xcs[g * GRP + s]
                    mean_f = wp.tile([128, CT], F32, tag="mean")
                    for t in range(2):
                        nc.scalar.activation(
                            junk[:], xca[:, t, :], ACTF.Copy,
                            scale=1.0 / HW, accum_out=mean_f[:, t:t + 1])
                    nc.vector.tensor_reduce(
                        mean_f[:, 2:4], xcb[:], axis=AX.X, op=ALU.add)
                    nc.vector.tensor_scalar_mul(
                        mean_f[:, 2:4], mean_f[:, 2:4], 1.0 / HW)
                    cls_bf = wp.tile([128, CT], BF16, tag="cls")
                    nc.vector.tensor_copy(cls_bf[:], mean_f[:])

                    # q_s = ISQ * Wq @ mean + qc
                    qm_ps = psQ.tile([128, CT, 33], F32, tag="qm")
                    for mc in range(CT):
                        for tk in range(CT):
                            nc.tensor.matmul(
                                qm_ps[:, mc, 0:1], wqt_s[:, tk, ts(mc, 128)],
                                cls_bf[:, tk:tk + 1],
                                start=(tk == 0), stop=(tk == CT - 1))
                    q_sb = wp.tile([128, CT], F32, tag="qsb")
                    nc.vector.scalar_tensor_tensor(
                        out=q_sb[:],
                        in0=qm_ps[:, :, 0:1].rearrange("p t one -> p (t one)"),
                        scalar=ISQ, in1=qc_s[:],
                        op0=ALU.mult, op1=ALU.add)
                    qblk = wp.tile([128, CT, 32], BF16, tag="qblk")
                    nc.vector.tensor_mul(
                        qblk[:],
                        q_sb[:, :, None].broadcast_to([128, CT, 32]),
                        mask_s[:])

                    # m_s = Wk.T @ qblk_s
                    for mc in range(CT):
                        for tk in range(CT):
                            nc.tensor.matmul(
                                qm_ps[:, mc, 1:33], wk_s[:, tk, ts(mc, 128)],
                                qblk[:, tk, :],
                                start=(tk == 0), stop=(tk == CT - 1))
                    m_sb = wp.tile([128, CT, 32], BF16, tag="msb")
                    nc.vector.tensor_copy(m_sb[:], qm_ps[:, :, 1:33])

                    # token + cls scores (pos folded into x on host)
                    for tk in range(CT):
                        xh = xca if tk < 2 else xcb
                        for jc in range(2):
                            nc.tensor.matmul(
                                sc_ps[32 * s:32 * s + 8, jc, :],
                                m_sb[:, tk, 0:8],
                                xh[:, tk % 2, ts(jc, 512)],
                                start=(tk == 0), stop=(tk == CT - 1),
                                tile_position=(0, 32 * s),
                                skip_group_check=True)
                        nc.tensor.matmul(
                            sc_ps[32 * s:32 * s + 8, 2, 0:1],
                            m_sb[:, tk, 0:8], pos0col_s[:, tk:tk + 1],
                            start=(tk == 0), stop=False,
                            tile_position=(0, 32 * s),
                            skip_group_check=True)
                        nc.tensor.matmul(
                            sc_ps[32 * s:32 * s + 8, 2, 0:1],
                            m_sb[:, tk, 0:8], cls_bf[:, tk:tk + 1],
                            start=False, stop=(tk == CT - 1),
                            tile_position=(0, 32 * s),
                            skip_group_check=True)
                return sc_ps

            def phase_b_slot(g, s, sc_ps, p_sb, rz, pclsf, tp_ps, pT_sb):
                """Per-slot softmax (32-row band) + pT transpose."""
                b0, b1 = 32 * s, 32 * s + 32
                sumexp = wp.tile([128, 1], F32, tag="sumexp", bufs=4)
                se2 = wp.tile([128, 1], F32, tag="se2", bufs=4)
                nc.scalar.activation(p_sb[b0:b1, :], sc_ps[b0:b1, 0:2, :],
                                     ACTF.Exp, scale=1.0,
                                     accum_out=sumexp[b0:b1, :])
                nc.scalar.activation(pclsf[b0:b1, :], sc_ps[b0:b1, 2, 0:1],
                                     ACTF.Exp, scale=1.0,
                                     accum_out=se2[b0:b1, :])
                nc.vector.tensor_add(sumexp[b0:b1, :], sumexp[b0:b1, :],
                                     se2[b0:b1, :])
                nc.vector.reciprocal(rz[b0:b1, :], sumexp[b0:b1, :])
                pcls_sc = wp.tile([128, 1], F32, tag="pclssc", bufs=4)
                nc.vector.tensor_scalar_mul(pcls_sc[b0:b1, :], pclsf[b0:b1, :],
                                            1.0 / HW)
                nc.vector.tensor_scalar_add(p_sb[b0:b1, :], p_sb[b0:b1, :],
                                            pcls_sc[b0:b1, :])
                for jc in range(JT):
                    nc.tensor.transpose(
                        tp_ps[:, jc, b0:b1], p_sb[b0:b1, ts(jc, 128)],
                        id32_s[b0:b1, :], tile_position=(b0, 0))
                nc.scalar.copy(pT_sb[:, :, b0:b1], tp_ps[:, :, b0:b1])

            def phase_c_slot(g, s, w_ps, rz, pclsf, pT_sb, w_sb):
                """Per-slot weighted sum + normalize into w_sb band."""
                b0, b1 = 32 * s, 32 * s + 32
                xta, xtb = xts[g * GRP + s]
                for jc in range(JT):
                    xh = xta if jc < 4 else xtb
                    nc.tensor.matmul(
                        w_ps[b0:b1, :], pT_sb[:, jc, b0:b1],
                        xh[:, jc % 4, :],
                        start=(jc == 0), stop=(jc == JT - 1),
                        tile_position=(0, b0),
                        skip_group_check=True)
                nc.vector.scalar_tensor_tensor(
                    out=w_ps[b0:b1, :], in0=pos0bc_s[b0:b1, :],
                    scalar=pclsf[b0:b1, :], in1=w_ps[b0:b1, :],
                    op0=ALU.mult, op1=ALU.add)
                nc.vector.tensor_scalar_mul(w_sb[b0:b1, :], w_ps[b0:b1, :],
                                            rz[b0:b1, :])

            def phase_fin(g, w_sb):
                """wT -> out projection -> bias -> store for a group."""
                wt_ps = psT.tile([128, CT, 128], BF16, tag="wt")
                for mc in range(CT):
                    nc.tensor.transpose(wt_ps[:, mc, :], w_sb[:, ts(mc, 128)],
                                        ident_s[:])
                wt_sb = wp.tile([128, CT, 128], BF16, tag="wtsb")
                nc.scalar.copy(wt_sb[:], wt_ps[:])
                out_ps = psQ.tile([128, CT, GRP], F32, tag="qm")
                for h in range(NH):
                    pr, hi = h // 2, 64 * (h % 2)
                    for tk in range(CT):
                        nc.tensor.matmul(
                            out_ps[hi:hi + 64, pr, :],
                            wvt_s[:, tk, h * DH:(h + 1) * DH],
                            wt_sb[:, tk, h::32],
                            start=(tk == 0), stop=(tk == CT - 1),
                            tile_position=(0, hi),
                            skip_group_check=True)
                out_sb = wp.tile([128, CT, GRP], F32, tag="outsb")
                for pr in range(CT):
                    nc.vector.tensor_scalar_add(out_sb[:, pr, :],
                                                out_ps[:, pr, :],
                                                bv_s[:, pr:pr + 1])
                nc.sync.dma_start(out=out_d[:, g], in_=out_sb[:])

            # ---------------- schedule: AB(g0), AB(g1), C(g1), C(g0) -------
            def phase_ab(g):
                sc_ps = phase_a(g)
                p_sb = wp.tile([128, HW], BF16, tag="psb")
                rz = wp.tile([128, 1], F32, tag="rz")
                pclsf = wp.tile([128, 1], F32, tag="pclsf")
                tp_ps = psT.tile([128, JT, 128], BF16, tag="tp")
                pT_sb = wp.tile([128, JT, 128], BF16, tag="pT")
                for s in range(GRP):
                    phase_b_slot(g, s, sc_ps, p_sb, rz, pclsf, tp_ps, pT_sb)
                return rz, pclsf, pT_sb

            def phase_c(g, rz, pclsf, pT_sb):
                w_ps = psW.tile([128, 512], F32, tag="w")
                w_sb = wp.tile([128, 512], BF16, tag="wsb")
                for s in range(GRP):
                    phase_c_slot(g, s, w_ps, rz, pclsf, pT_sb, w_sb)
                phase_fin(g, w_sb)

            b0 = phase_ab(0)
            b1 = phase_ab(1)
            phase_c(1, *b1)
            phase_c(0, *b0)

    nc.compile()
    return nc


def _prep(pos_emb, Wq, bq, Wk, bk, Wv, bv):
    import ml_dtypes
    bf = ml_dtypes.bfloat16

    def ptn(v):  # [512] -> [128, CT], c = t*128 + p
        return np.ascontiguousarray(v.reshape(CT, 128).T)

    def chunkk(w):  # [512, N] -> [128, CT, N], k = t*128 + p
        return np.ascontiguousarray(w.reshape(CT, 128, -1).transpose(1, 0, 2))

    p1 = pos_emb[1:].sum(axis=0)
    pos0adj = (pos_emb[0] - p1 / HW).astype(np.float32)
    qcv = ISQ * (Wq.astype(np.float64) @ pos0adj.astype(np.float64)
                 + bq.astype(np.float64)).astype(np.float32)
    mask = np.zeros((128, CT, 32), np.float32)
    for p in range(128):
        for t in range(CT):
            h = (t * 128 + p) // DH
            mask[p, t, h] = 1.0

    return {
        "wqt": chunkk(np.ascontiguousarray(Wq.T)).astype(bf),
        "wk": chunkk(Wk).astype(bf),
        "wvt": chunkk(np.ascontiguousarray(Wv.T)).astype(bf),
        "pos0col": ptn(pos0adj).astype(bf),
        "pos0bc": np.ascontiguousarray(
            np.broadcast_to(pos0adj.reshape(1, C), (128, C))).astype(bf),
        "qc": ptn(qcv),
        "bv": ptn(bv),
        "mask32": mask.astype(bf),
        "ident": np.eye(128, dtype=np.float32).astype(bf),
        "ident32": np.tile(np.eye(32, dtype=np.float32), (4, 1)).astype(bf),
    }


def _prep_x(x, pos_emb):
    """(x + pos) [B, C, HW] -> per-core (xc, xt) bf16 DMA-friendly layouts."""
    import ml_dtypes
    bf = ml_dtypes.bfloat16
    xb = (x + pos_emb[1:].T[None]).astype(bf)
    xcs, xts = [], []
    for i in range(NCORES):
        xs = xb[i * BPC:(i + 1) * BPC]                       # [BPC, C, HW]
        xc = xs.reshape(BPC, CT, 128, HW).transpose(0, 2, 1, 3)
        xcs.append(np.ascontiguousarray(xc))                 # [BPC,128,CT,HW]
        xt = xs.transpose(0, 2, 1).reshape(BPC, JT, 128, C)
        xt = xt.transpose(0, 2, 1, 3)
        xts.append(np.ascontiguousarray(xt))                 # [BPC,128,JT,C]
    return xcs, xts


def _in_maps(x, pos_emb, Wq, bq, Wk, bk, Wv, bv):
    shared = _prep(np.asarray(pos_emb, np.float32), np.asarray(Wq, np.float32),
                   np.asarray(bq, np.float32), np.asarray(Wk, np.float32),
                   np.asarray(bk, np.float32), np.asarray(Wv, np.float32),
                   np.asarray(bv, np.float32))
    xcs, xts = _prep_x(np.asarray(x, np.float32).reshape(B, C, HW),
                       np.asarray(pos_emb, np.float32))
    in_maps = []
    for i in range(NCORES):
        m = dict(shared)
        m["xc"] = xcs[i]
        m["xt"] = xts[i]
        in_maps.append(m)
    return in_maps


def kernel(x, pos_emb, Wq, bq, Wk, bk, Wv, bv, num_heads):
    assert int(num_heads) == NH
    if "nc" not in _CACHE:
        _CACHE["nc"] = _build_nc()
    nc = _CACHE["nc"]
    in_maps = _in_maps(x, pos_emb, Wq, bq, Wk, bk, Wv, bv)
    res = run_bass_kernel_spmd(nc, in_maps, list(range(NCORES)))
    # out_d [128, NGRP, CT, GRP] -> [BPC, C]
    outs = []
    for i in range(NCORES):
        v = res.results[i]["out"]                 # [128, NGRP, CT, GRP]
        o = v.transpose(1, 3, 2, 0).reshape(BPC, C)   # [g, s, t, p] -> b, c
        outs.append(o)
    return np.concatenate(outs, axis=0).astype(np.float32)


# revision 19
# speedup vs baseline: 1.2402x; 1.2402x over previous
"""CLS-AttentionPool2d Trainium2 kernel (8 NeuronCores, data-parallel over batch).

v4: host folds pos_emb into x (both layouts, bf16) so scores/weighted-sums
need no separate kp/postok tensors; all stationaries bf16 (FWL). DMA order
params -> xc g0 -> xc g1 -> xt g0 -> params2 -> xt g1 on the sync HWDGE ring
so both groups' score chains run during the xt stream; program phases
interleaved A0 B0 A1 B1 C0 C1. PE warmup burst beats the HAM cold clock.

Math (single CLS query => tiny attention):
  xp        = x + pos[1:]^T (host)
  mean      = xp.mean(j)  (= mean_x + posmean)
  q  = ISQ*Wq@mean + qc,  qc = ISQ*(Wq@pos0adj + bq),  pos0adj = pos0-posmean
  qblk[k, u] = q[k] * [head(k) == u]
  m  = Wk.T @ qblk
  scores[(s,h), j] = m.T @ xp ;  scores[., cls] = rowmean(scores) + m.T@pos0adj
  p = softmax(scores);  p' = p_tok + p_cls/1024
  w  = (p'.T @ xp^T + p_cls * pos0adj) / Z
  out = Wv @ w + bv   (per-head block of Wv)
"""

import math
import numpy as np

import concourse.bass as bass
import concourse.mybir as mybir
import concourse.tile as tile
from concourse import bacc
from concourse.bass import ts
from concourse.bass_utils import run_bass_kernel_spmd

F32 = mybir.dt.float32
BF16 = mybir.dt.bfloat16
AX = mybir.AxisListType
ALU = mybir.AluOpType
ACTF = mybir.ActivationFunctionType

B, C, HW = 64, 512, 1024
NH, DH = 8, 64
NCORES = 8
BPC = B // NCORES          # 8 batches per core
GRP = 4                    # batches per group (2 groups per core)
NGRP = BPC // GRP
CT = C // 128              # 4 c-chunks
JT = HW // 128             # 8 j-chunks
ISQ = 1.0 / math.sqrt(C)

_CACHE = {}


def _build_nc():
    nc = bacc.Bacc("TRN2", target_bir_lowering=False, debug=False,
                   num_devices=NCORES)

    # ---- DRAM I/O ----
    xc_d = nc.dram_tensor("xc", [BPC, 128, CT, HW], BF16, kind="ExternalInput")
    xt_d = nc.dram_tensor("xt", [BPC, 128, JT, C], BF16, kind="ExternalInput")
    wqt = nc.dram_tensor("wqt", [128, CT, C], BF16, kind="ExternalInput")
    wk = nc.dram_tensor("wk", [128, CT, C], BF16, kind="ExternalInput")
    wvt = nc.dram_tensor("wvt", [128, CT, C], BF16, kind="ExternalInput")
    pos0col = nc.dram_tensor("pos0col", [128, CT], BF16, kind="ExternalInput")
    pos0bc = nc.dram_tensor("pos0bc", [128, C], BF16, kind="ExternalInput")
    qc = nc.dram_tensor("qc", [128, CT], F32, kind="ExternalInput")
    bv = nc.dram_tensor("bv", [128, CT], F32, kind="ExternalInput")
    mask32 = nc.dram_tensor("mask32", [128, CT, 32], BF16, kind="ExternalInput")
    ident = nc.dram_tensor("ident", [128, 128], BF16, kind="ExternalInput")
    ident32 = nc.dram_tensor("ident32", [128, 32], BF16, kind="ExternalInput")
    out_d = nc.dram_tensor("out", [128, NGRP, CT, GRP], F32,
                           kind="ExternalOutput")

    with tile.TileContext(nc) as tc:
        with (
            tc.tile_pool(name="persist", bufs=1) as pp,
            tc.tile_pool(name="work", bufs=2) as wp,
            tc.tile_pool(name="psSC", bufs=1, space="PSUM") as psSC,
            tc.tile_pool(name="psW", bufs=1, space="PSUM") as psW,
            tc.tile_pool(name="psT", bufs=1, space="PSUM") as psT,
            tc.tile_pool(name="psQ", bufs=2, space="PSUM") as psQ,
        ):
            # ---- persistent tiles ----
            ident_s = pp.tile([128, 128], BF16)
            id32_s = pp.tile([128, 32], BF16)
            wk_s = pp.tile([128, CT, C], BF16)
            wqt_s = pp.tile([128, CT, C], BF16)
            qc_s = pp.tile([128, CT], F32)
            mask_s = pp.tile([128, CT, 32], BF16)
            pos0col_s = pp.tile([128, CT], BF16)
            pos0bc_s = pp.tile([128, C], BF16)
            wvt_s = pp.tile([128, CT, C], BF16)
            bv_s = pp.tile([128, CT], F32)
            xcs = [[pp.tile([128, 2, HW], BF16, name=f"xc{i}h{h}")
                    for h in range(2)] for i in range(BPC)]
            xts = [[pp.tile([128, 4, C], BF16, name=f"xt{i}h{h}")
                    for h in range(2)] for i in range(BPC)]
            junk = pp.tile([128, HW], BF16)

            # ---- ALL load DMAs up-front on the sync HWDGE ring (FIFO):
            # params -> xc g0 -> xc g1 -> xt g0 -> params2 -> xt g1
            for dst, src_ in [(ident_s, ident), (id32_s, ident32),
                              (wk_s, wk), (wqt_s, wqt), (qc_s, qc),
                              (mask_s, mask32), (pos0col_s, pos0col)]:
                nc.sync.dma_start(out=dst[:], in_=src_[:])
            for s in range(BPC):
                for h in range(2):
                    nc.sync.dma_start(out=xcs[s][h][:],
                                      in_=xc_d[s][:, 2 * h:2 * h + 2, :])
            for s in range(GRP):
                for h in range(2):
                    nc.sync.dma_start(out=xts[s][h][:],
                                      in_=xt_d[s][:, 4 * h:4 * h + 4, :])
            nc.sync.dma_start(out=wvt_s[:], in_=wvt[:])
            nc.sync.dma_start(out=bv_s[:], in_=bv[:])
            nc.sync.dma_start(out=pos0bc_s[:], in_=pos0bc[:])
            for s in range(GRP, BPC):
                for h in range(2):
                    nc.sync.dma_start(out=xts[s][h][:],
                                      in_=xt_d[s][:, 4 * h:4 * h + 4, :])

            # ---- PE warmup burst (HAM unthrottle) while DMAs stream ----
            warm_ps = psT.tile([128, JT, 128], BF16, tag="tp")
            for _ in range(48):
                nc.tensor.transpose(warm_ps[:, 0, :], ident_s[:], ident_s[:])

            # ---------------- phase bodies ----------------
            def phase_a(g):
                """Per-slot: mean -> q -> qblk -> m -> token+cls scores."""
                sc_ps = psSC.tile([128, 3, 512], F32, tag="sc")
                for s in range(GRP):
                    xca, xcb = xcs[g * GRP + s]
                    mean_f = wp.tile([128, CT], F32, tag="mean")
                    for t in range(2):
                        nc.scalar.activation(
                            junk[:], xca[:, t, :], ACTF.Copy,
                            scale=1.0 / HW, accum_out=mean_f[:, t:t + 1])
                    nc.vector.tensor_reduce(
                        mean_f[:, 2:4], xcb[:], axis=AX.X, op=ALU.add)
                    nc.vector.tensor_scalar_mul(
                        mean_f[:, 2:4], mean_f[:, 2:4], 1.0 / HW)
                    cls_bf = wp.tile([128, CT], BF16, tag="cls")
                    nc.vector.tensor_copy(cls_bf[:], mean_f[:])

                    # q_s = ISQ * Wq @ mean + qc
                    qm_ps = psQ.tile([128, CT, 33], F32, tag="qm")
                    for mc in range(CT):
                        for tk in range(CT):
                            nc.tensor.matmul(
                                qm_ps[:, mc, 0:1], wqt_s[:, tk, ts(mc, 128)],
                                cls_bf[:, tk:tk + 1],
                                start=(tk == 0), stop=(tk == CT - 1))
                    q_sb = wp.tile([128, CT], F32, tag="qsb")
                    nc.vector.scalar_tensor_tensor(
                        out=q_sb[:],
                        in0=qm_ps[:, :, 0:1].rearrange("p t one -> p (t one)"),
                        scalar=ISQ, in1=qc_s[:],
                        op0=ALU.mult, op1=ALU.add)
                    qblk = wp.tile([128, CT, 32], BF16, tag="qblk")
                    nc.vector.tensor_mul(
                        qblk[:],
                        q_sb[:, :, None].broadcast_to([128, CT, 32]),
                        mask_s[:])

                    # m_s = Wk.T @ qblk_s
                    for mc in range(CT):
                        for tk in range(CT):
                            nc.tensor.matmul(
                                qm_ps[:, mc, 1:33], wk_s[:, tk, ts(mc, 128)],
                                qblk[:, tk, :],
                                start=(tk == 0), stop=(tk == CT - 1))
                    m_sb = wp.tile([128, CT, 32], BF16, tag="msb")
                    nc.vector.tensor_copy(m_sb[:], qm_ps[:, :, 1:33])

                    # token + cls scores (pos folded into x on host)
                    for tk in range(CT):
                        xh = xca if tk < 2 else xcb
                        for jc in range(2):
                            nc.tensor.matmul(
                                sc_ps[32 * s:32 * s + 8, jc, :],
                                m_sb[:, tk, 0:8],
                                xh[:, tk % 2, ts(jc, 512)],
                                start=(tk == 0), stop=(tk == CT - 1),
                                tile_position=(0, 32 * s),
                                skip_group_check=True)
                        nc.tensor.matmul(
                            sc_ps[32 * s:32 * s + 8, 2, 0:1],
                            m_sb[:, tk, 0:8], pos0col_s[:, tk:tk + 1],
                            start=(tk == 0), stop=(tk == CT - 1),
                            tile_position=(0, 32 * s),
                            skip_group_check=True)
                return sc_ps

            def phase_b(sc_ps):
                """Softmax + per-slot pT transposes."""
                # CLS col: += rowmean of token scores
                redcol = wp.tile([128, 1], F32, tag="redcol")
                nc.vector.reduce_sum(redcol[:], sc_ps[:, 0:2, :], axis=AX.XY)
                nc.vector.scalar_tensor_tensor(
                    out=sc_ps[:, 2, 0:1], in0=redcol[:], scalar=1.0 / HW,
                    in1=sc_ps[:, 2, 0:1], op0=ALU.mult, op1=ALU.add)

                p_sb = wp.tile([128, HW], BF16, tag="psb")
                sumexp = wp.tile([128, 1], F32, tag="sumexp")
                se2 = wp.tile([128, 1], F32, tag="se2")
                pclsf = wp.tile([128, 1], F32, tag="pclsf")
                nc.scalar.activation(p_sb[:], sc_ps[:, 0:2, :], ACTF.Exp,
                                     scale=1.0, accum_out=sumexp[:])
                nc.scalar.activation(pclsf[:], sc_ps[:, 2, 0:1],
                                     ACTF.Exp, scale=1.0, accum_out=se2[:])
                nc.vector.tensor_add(sumexp[:], sumexp[:], se2[:])
                rz = wp.tile([128, 1], F32, tag="rz")
                nc.vector.reciprocal(rz[:], sumexp[:])
                # p' : fold CLS-mean into token weights
                pcls_sc = wp.tile([128, 1], F32, tag="pclssc")
                nc.vector.tensor_scalar_mul(pcls_sc[:], pclsf[:], 1.0 / HW)
                nc.vector.tensor_scalar_add(p_sb[:], p_sb[:], pcls_sc[:])

                tp_ps = psT.tile([128, JT, 128], BF16, tag="tp")
                pT_sb = wp.tile([128, JT, 128], BF16, tag="pT")
                for s in range(GRP):
                    for jc in range(JT):
                        nc.tensor.transpose(
                            tp_ps[:, jc, 32 * s:32 * s + 32],
                            p_sb[32 * s:32 * s + 32, ts(jc, 128)],
                            id32_s[32 * s:32 * s + 32, :],
                            tile_position=(32 * s, 0))
                    nc.vector.tensor_copy(pT_sb[:, :, 32 * s:32 * s + 32],
                                          tp_ps[:, :, 32 * s:32 * s + 32])
                return p_sb, rz, pclsf, pT_sb

            def phase_c(g, rz, pclsf, pT_sb):
                """Per-slot weighted sums -> w -> wT -> out projection."""
                w_ps = psW.tile([128, 512], F32, tag="w")
                for s in range(GRP):
                    xta, xtb = xts[g * GRP + s]
                    for jc in range(JT):
                        xh = xta if jc < 4 else xtb
                        nc.tensor.matmul(
                            w_ps[32 * s:32 * s + 32, :],
                            pT_sb[:, jc, 32 * s:32 * s + 32],
                            xh[:, jc % 4, :],
                            start=(jc == 0), stop=(jc == JT - 1),
                            tile_position=(0, 32 * s),
                            skip_group_check=True)
                # w_ps += p_cls * pos0adj (f32), then w = w_ps * rz
                nc.vector.scalar_tensor_tensor(
                    out=w_ps[:], in0=pos0bc_s[:], scalar=pclsf[:],
                    in1=w_ps[:], op0=ALU.mult, op1=ALU.add)
                w_sb = wp.tile([128, 512], BF16, tag="wsb")
                nc.vector.tensor_scalar_mul(w_sb[:], w_ps[:], rz[:])

                # wT via PE transpose
                wt_ps = psT.tile([128, CT, 128], BF16, tag="wt")
                for mc in range(CT):
                    nc.tensor.transpose(wt_ps[:, mc, :], w_sb[:, ts(mc, 128)],
                                        ident_s[:])
                wt_sb = wp.tile([128, CT, 128], BF16, tag="wtsb")
                nc.vector.tensor_copy(wt_sb[:], wt_ps[:])

                # output projection
                wv2 = psW.tile([128, 512], F32, tag="w")
                out_ps = wv2[:, 0:16].rearrange("p (t s) -> p t s", t=CT)
                for h in range(NH):
                    pr, hi = h // 2, 64 * (h % 2)
                    for tk in range(CT):
                        nc.tensor.matmul(
                            out_ps[hi:hi + 64, pr, :],
                            wvt_s[:, tk, h * DH:(h + 1) * DH],
                            wt_sb[:, tk, h::32],
                            start=(tk == 0), stop=(tk == CT - 1),
                            tile_position=(0, hi),
                            skip_group_check=True)
                out_sb = wp.tile([128, CT, GRP], F32, tag="outsb")
                for pr in range(CT):
                    nc.vector.tensor_scalar_add(out_sb[:, pr, :],
                                                out_ps[:, pr, :],
                                                bv_s[:, pr:pr + 1])
                nc.sync.dma_start(out=out_d[:, g], in_=out_sb[:])

            # ---------------- interleaved schedule ----------------
            sc0 = phase_a(0)
            b0 = phase_b(sc0)
            sc1 = phase_a(1)
            b1 = phase_b(sc1)
            phase_c(0, b0[1], b0[2], b0[3])
            phase_c(1, b1[1], b1[2], b1[3])

    nc.compile()
    return nc


def _prep(pos_emb, Wq, bq, Wk, bk, Wv, bv):
    import ml_dtypes
    bf = ml_dtypes.bfloat16

    def ptn(v):  # [512] -> [128, CT], c = t*128 + p
        return np.ascontiguousarray(v.reshape(CT, 128).T)

    def chunkk(w):  # [512, N] -> [128, CT, N], k = t*128 + p
        return np.ascontiguousarray(w.reshape(CT, 128, -1).transpose(1, 0, 2))

    p1 = pos_emb[1:].sum(axis=0)
    pos0adj = (pos_emb[0] - p1 / HW).astype(np.float32)
    qcv = ISQ * (Wq.astype(np.float64) @ pos0adj.astype(np.float64)
                 + bq.astype(np.float64)).astype(np.float32)
    mask = np.zeros((128, CT, 32), np.float32)
    for p in range(128):
        for t in range(CT):
            h = (t * 128 + p) // DH
            mask[p, t, h] = 1.0

    return {
        "wqt": chunkk(np.ascontiguousarray(Wq.T)).astype(bf),
        "wk": chunkk(Wk).astype(bf),
        "wvt": chunkk(np.ascontiguousarray(Wv.T)).astype(bf),
        "pos0col": ptn(pos0adj).astype(bf),
        "pos0bc": np.ascontiguousarray(
            np.broadcast_to(pos0adj.reshape(1, C), (128, C))).astype(bf),
        "qc": ptn(qcv),
        "bv": ptn(bv),
        "mask32": mask.astype(bf),
        "ident": np.eye(128, dtype=np.float32).astype(bf),
        "ident32": np.tile(np.eye(32, dtype=np.float32), (4, 1)).astype(bf),
    }


def _prep_x(x, pos_emb):
    """(x + pos) [B, C, HW] -> per-core (xc, xt) bf16 DMA-friendly layouts."""
    import ml_dtypes
    bf = ml_dtypes.bfloat16
    xb = (x + pos_emb[1:].T[None]).astype(bf)
    xcs, xts = [], []
    for i in range(NCORES):
        xs = xb[i * BPC:(i + 1) * BPC]                       # [BPC, C, HW]
        xc = xs.reshape(BPC, CT, 128, HW).transpose(0, 2, 1, 3)
        xcs.append(np.ascontiguousarray(xc))                 # [BPC,128,CT,HW]
        xt = xs.transpose(0, 2, 1).reshape(BPC, JT, 128, C)
        xt = xt.transpose(0, 2, 1, 3)
        xts.append(np.ascontiguousarray(xt))                 # [BPC,128,JT,C]
    return xcs, xts


def _in_maps(x, pos_emb, Wq, bq, Wk, bk, Wv, bv):
    shared = _prep(np.asarray(pos_emb, np.float32), np.asarray(Wq, np.float32),
                   np.asarray(bq, np.float32), np.asarray(Wk, np.float32),
                   np.asarray(bk, np.float32), np.asarray(Wv, np.float32),
                   np.asarray(bv, np.float32))
    xcs, xts = _prep_x(np.asarray(x, np.float32).reshape(B, C, HW),
                       np.asarray(pos_emb, np.float32))
    in_maps = []
    for i in range(NCORES):
        m = dict(shared)
        m["xc"] = xcs[i]
        m["xt"] = xts[i]
        in_maps.append(m)
    return in_maps


def kernel(x, pos_emb, Wq, bq, Wk, bk, Wv, bv, num_heads):
    assert int(num_heads) == NH
    if "nc" not in _CACHE:
        _CACHE["nc"] = _build_nc()
    nc = _CACHE["nc"]
    in_maps = _in_maps(x, pos_emb, Wq, bq, Wk, bk, Wv, bv)
    res = run_bass_kernel_spmd(nc, in_maps, list(range(NCORES)))
    # out_d [128, NGRP, CT, GRP] -> [BPC, C]
    outs = []
    for i in range(NCORES):
        v = res.results[i]["out"]                 # [128, NGRP, CT, GRP]
        o = v.transpose(1, 3, 2, 0).reshape(BPC, C)   # [g, s, t, p] -> b, c
        outs.append(o)
    return np.concatenate(outs, axis=0).astype(np.float32)


# revision 21
# speedup vs baseline: 1.4005x; 1.1292x over previous
"""CLS-AttentionPool2d Trainium2 kernel (8 NeuronCores, data-parallel over batch).

v4: host folds pos_emb into x (both layouts, bf16) so scores/weighted-sums
need no separate kp/postok tensors; all stationaries bf16 (FWL). DMA order
params -> xc g0 -> xc g1 -> xt g0 -> params2 -> xt g1 on the sync HWDGE ring
so both groups' score chains run during the xt stream; program phases
interleaved A0 B0 A1 B1 C0 C1. PE warmup burst beats the HAM cold clock.

Math (single CLS query => tiny attention):
  xp        = x + pos[1:]^T (host)
  mean      = xp.mean(j)  (= mean_x + posmean)
  q  = ISQ*Wq@mean + qc,  qc = ISQ*(Wq@pos0adj + bq),  pos0adj = pos0-posmean
  qblk[k, u] = q[k] * [head(k) == u]
  m  = Wk.T @ qblk
  scores[(s,h), j] = m.T @ xp ;  scores[., cls] = rowmean(scores) + m.T@pos0adj
  p = softmax(scores);  p' = p_tok + p_cls/1024
  w  = (p'.T @ xp^T + p_cls * pos0adj) / Z
  out = Wv @ w + bv   (per-head block of Wv)
"""

import math
import numpy as np

import concourse.bass as bass
import concourse.mybir as mybir
import concourse.tile as tile
from concourse import bacc
from concourse.bass import ts
from concourse.bass_utils import run_bass_kernel_spmd

F32 = mybir.dt.float32
BF16 = mybir.dt.bfloat16
AX = mybir.AxisListType
ALU = mybir.AluOpType
ACTF = mybir.ActivationFunctionType

B, C, HW = 64, 512, 1024
NH, DH = 8, 64
NCORES = 8
BPC = B // NCORES          # 8 batches per core
GRP = 4                    # batches per group (2 groups per core)
NGRP = BPC // GRP
CT = C // 128              # 4 c-chunks
JT = HW // 128             # 8 j-chunks
ISQ = 1.0 / math.sqrt(C)

_CACHE = {}


def _build_nc():
    nc = bacc.Bacc("TRN2", target_bir_lowering=False, debug=False,
                   num_devices=NCORES)

    # ---- DRAM I/O ----
    xc_d = nc.dram_tensor("xc", [BPC, 128, CT, HW], BF16, kind="ExternalInput")
    xt_d = nc.dram_tensor("xt", [BPC, 128, JT, C], BF16, kind="ExternalInput")
    wqt = nc.dram_tensor("wqt", [128, CT, C], BF16, kind="ExternalInput")
    wk = nc.dram_tensor("wk", [128, CT, C], BF16, kind="ExternalInput")
    wvt = nc.dram_tensor("wvt", [128, CT, C], BF16, kind="ExternalInput")
    pos0col = nc.dram_tensor("pos0col", [128, CT], BF16, kind="ExternalInput")
    pos0bc = nc.dram_tensor("pos0bc", [128, C], BF16, kind="ExternalInput")
    qc = nc.dram_tensor("qc", [128, CT], F32, kind="ExternalInput")
    bv = nc.dram_tensor("bv", [128, CT], F32, kind="ExternalInput")
    mask32 = nc.dram_tensor("mask32", [128, CT, 32], BF16, kind="ExternalInput")
    ident = nc.dram_tensor("ident", [128, 128], BF16, kind="ExternalInput")
    ident32 = nc.dram_tensor("ident32", [128, 32], BF16, kind="ExternalInput")
    out_d = nc.dram_tensor("out", [128, NGRP, CT, GRP], F32,
                           kind="ExternalOutput")

    with tile.TileContext(nc) as tc:
        with (
            tc.tile_pool(name="persist", bufs=1) as pp,
            tc.tile_pool(name="work", bufs=2) as wp,
            tc.tile_pool(name="psSC", bufs=1, space="PSUM") as psSC,
            tc.tile_pool(name="psW", bufs=1, space="PSUM") as psW,
            tc.tile_pool(name="psT", bufs=1, space="PSUM") as psT,
            tc.tile_pool(name="psQ", bufs=2, space="PSUM") as psQ,
        ):
            # ---- persistent tiles ----
            ident_s = pp.tile([128, 128], BF16)
            id32_s = pp.tile([128, 32], BF16)
            wk_s = pp.tile([128, CT, C], BF16)
            wqt_s = pp.tile([128, CT, C], BF16)
            qc_s = pp.tile([128, CT], F32)
            mask_s = pp.tile([128, CT, 32], BF16)
            pos0col_s = pp.tile([128, CT], BF16)
            pos0bc_s = pp.tile([128, C], BF16)
            wvt_s = pp.tile([128, CT, C], BF16)
            bv_s = pp.tile([128, CT], F32)
            xcs = [[pp.tile([128, 2, HW], BF16, name=f"xc{i}h{h}")
                    for h in range(2)] for i in range(BPC)]
            xts = [[pp.tile([128, 4, C], BF16, name=f"xt{i}h{h}")
                    for h in range(2)] for i in range(BPC)]
            junk = pp.tile([128, HW], BF16)

            # ---- ALL load DMAs up-front on the sync HWDGE ring (FIFO):
            # params -> xc g0 -> xc g1 -> xt g0 -> params2 -> xt g1
            for dst, src_ in [(ident_s, ident), (id32_s, ident32),
                              (wk_s, wk), (wqt_s, wqt), (qc_s, qc),
                              (mask_s, mask32), (pos0col_s, pos0col)]:
                nc.sync.dma_start(out=dst[:], in_=src_[:])
            for s in range(BPC):
                for h in range(2):
                    nc.sync.dma_start(out=xcs[s][h][:],
                                      in_=xc_d[s][:, 2 * h:2 * h + 2, :])
            for s in range(GRP):
                for h in range(2):
                    nc.sync.dma_start(out=xts[s][h][:],
                                      in_=xt_d[s][:, 4 * h:4 * h + 4, :])
            nc.sync.dma_start(out=wvt_s[:], in_=wvt[:])
            nc.sync.dma_start(out=bv_s[:], in_=bv[:])
            nc.sync.dma_start(out=pos0bc_s[:], in_=pos0bc[:])
            for s in range(GRP, BPC):
                for h in range(2):
                    nc.sync.dma_start(out=xts[s][h][:],
                                      in_=xt_d[s][:, 4 * h:4 * h + 4, :])

            # ---- PE warmup burst (HAM unthrottle) while DMAs stream ----
            warm_ps = psT.tile([128, JT, 128], BF16, tag="tp")
            for _ in range(48):
                nc.tensor.transpose(warm_ps[:, 0, :], ident_s[:], ident_s[:])

            # ---------------- phase bodies ----------------
            def phase_a(g):
                """Per-slot: mean -> q -> qblk -> m -> token+cls scores."""
                sc_ps = psSC.tile([128, 3, 512], F32, tag="sc")
                for s in range(GRP):
                    xca, xcb = xcs[g * GRP + s]
                    mean_f = wp.tile([128, CT], F32, tag="mean")
                    for t in range(2):
                        nc.scalar.activation(
                            junk[:], xca[:, t, :], ACTF.Copy,
                            scale=1.0 / HW, accum_out=mean_f[:, t:t + 1])
                    nc.vector.tensor_reduce(
                        mean_f[:, 2:4], xcb[:], axis=AX.X, op=ALU.add)
                    nc.vector.tensor_scalar_mul(
                        mean_f[:, 2:4], mean_f[:, 2:4], 1.0 / HW)
                    cls_bf = wp.tile([128, CT], BF16, tag="cls")
                    nc.vector.tensor_copy(cls_bf[:], mean_f[:])

                    # q_s = ISQ * Wq @ mean + qc
                    qm_ps = psQ.tile([128, CT, 33], F32, tag="qm")
                    for mc in range(CT):
                        for tk in range(CT):
                            nc.tensor.matmul(
                                qm_ps[:, mc, 0:1], wqt_s[:, tk, ts(mc, 128)],
                                cls_bf[:, tk:tk + 1],
                                start=(tk == 0), stop=(tk == CT - 1))
                    q_sb = wp.tile([128, CT], F32, tag="qsb")
                    nc.vector.scalar_tensor_tensor(
                        out=q_sb[:],
                        in0=qm_ps[:, :, 0:1].rearrange("p t one -> p (t one)"),
                        scalar=ISQ, in1=qc_s[:],
                        op0=ALU.mult, op1=ALU.add)
                    qblk = wp.tile([128, CT, 32], BF16, tag="qblk")
                    nc.vector.tensor_mul(
                        qblk[:],
                        q_sb[:, :, None].broadcast_to([128, CT, 32]),
                        mask_s[:])

                    # m_s = Wk.T @ qblk_s
                    for mc in range(CT):
                        for tk in range(CT):
                            nc.tensor.matmul(
                                qm_ps[:, mc, 1:33], wk_s[:, tk, ts(mc, 128)],
                                qblk[:, tk, :],
                                start=(tk == 0), stop=(tk == CT - 1))
                    m_sb = wp.tile([128, CT, 32], BF16, tag="msb")
                    nc.vector.tensor_copy(m_sb[:], qm_ps[:, :, 1:33])

                    # token + cls scores (pos folded into x on host)
                    for tk in range(CT):
                        xh = xca if tk < 2 else xcb
                        for jc in range(2):
                            nc.tensor.matmul(
                                sc_ps[32 * s:32 * s + 8, jc, :],
                                m_sb[:, tk, 0:8],
                                xh[:, tk % 2, ts(jc, 512)],
                                start=(tk == 0), stop=(tk == CT - 1),
                                tile_position=(0, 32 * s),
                                skip_group_check=True)
                        nc.tensor.matmul(
                            sc_ps[32 * s:32 * s + 8, 2, 0:1],
                            m_sb[:, tk, 0:8], pos0col_s[:, tk:tk + 1],
                            start=(tk == 0), stop=(tk == CT - 1),
                            tile_position=(0, 32 * s),
                            skip_group_check=True)
                return sc_ps

            def phase_b_exp(sc_ps):
                """Softmax (ACT/DVE only - no PE work)."""
                # CLS col: += rowmean of token scores
                redcol = wp.tile([128, 1], F32, tag="redcol")
                nc.vector.reduce_sum(redcol[:], sc_ps[:, 0:2, :], axis=AX.XY)
                nc.vector.scalar_tensor_tensor(
                    out=sc_ps[:, 2, 0:1], in0=redcol[:], scalar=1.0 / HW,
                    in1=sc_ps[:, 2, 0:1], op0=ALU.mult, op1=ALU.add)

                p_sb = wp.tile([128, HW], BF16, tag="psb")
                sumexp = wp.tile([128, 1], F32, tag="sumexp")
                se2 = wp.tile([128, 1], F32, tag="se2")
                pclsf = wp.tile([128, 1], F32, tag="pclsf")
                nc.scalar.activation(p_sb[:], sc_ps[:, 0:2, :], ACTF.Exp,
                                     scale=1.0, accum_out=sumexp[:])
                nc.scalar.activation(pclsf[:], sc_ps[:, 2, 0:1],
                                     ACTF.Exp, scale=1.0, accum_out=se2[:])
                nc.vector.tensor_add(sumexp[:], sumexp[:], se2[:])
                rz = wp.tile([128, 1], F32, tag="rz")
                nc.vector.reciprocal(rz[:], sumexp[:])
                # p' : fold CLS-mean into token weights
                pcls_sc = wp.tile([128, 1], F32, tag="pclssc")
                nc.vector.tensor_scalar_mul(pcls_sc[:], pclsf[:], 1.0 / HW)
                nc.vector.tensor_scalar_add(p_sb[:], p_sb[:], pcls_sc[:])
                return p_sb, rz, pclsf

            def phase_b_tp(p_sb, rz, pclsf):
                """pT transposes (PE) + copies."""
                tp_ps = psT.tile([128, JT, 128], BF16, tag="tp")
                pT_sb = wp.tile([128, JT, 128], BF16, tag="pT")
                for s in range(GRP):
                    for jc in range(JT):
                        nc.tensor.transpose(
                            tp_ps[:, jc, 32 * s:32 * s + 32],
                            p_sb[32 * s:32 * s + 32, ts(jc, 128)],
                            id32_s[32 * s:32 * s + 32, :],
                            tile_position=(32 * s, 0))
                    nc.vector.tensor_copy(pT_sb[:, :, 32 * s:32 * s + 32],
                                          tp_ps[:, :, 32 * s:32 * s + 32])
                return p_sb, rz, pclsf, pT_sb

            def phase_c(g, rz, pclsf, pT_sb):
                """Per-slot weighted sums -> w -> wT -> out projection."""
                w_ps = psW.tile([128, 512], F32, tag="w")
                for s in range(GRP):
                    xta, xtb = xts[g * GRP + s]
                    for jc in range(JT):
                        xh = xta if jc < 4 else xtb
                        nc.tensor.matmul(
                            w_ps[32 * s:32 * s + 32, :],
                            pT_sb[:, jc, 32 * s:32 * s + 32],
                            xh[:, jc % 4, :],
                            start=(jc == 0), stop=(jc == JT - 1),
                            tile_position=(0, 32 * s),
                            skip_group_check=True)
                # w_ps += p_cls * pos0adj (f32), then w = w_ps * rz
                nc.vector.scalar_tensor_tensor(
                    out=w_ps[:], in0=pos0bc_s[:], scalar=pclsf[:],
                    in1=w_ps[:], op0=ALU.mult, op1=ALU.add)
                w_sb = wp.tile([128, 512], BF16, tag="wsb")
                nc.vector.tensor_scalar_mul(w_sb[:], w_ps[:], rz[:])

                # wT via PE transpose
                wt_ps = psT.tile([128, CT, 128], BF16, tag="wt")
                for mc in range(CT):
                    nc.tensor.transpose(wt_ps[:, mc, :], w_sb[:, ts(mc, 128)],
                                        ident_s[:])
                wt_sb = wp.tile([128, CT, 128], BF16, tag="wtsb")
                nc.vector.tensor_copy(wt_sb[:], wt_ps[:])

                # output projection
                wv2 = psW.tile([128, 512], F32, tag="w")
                out_ps = wv2[:, 0:16].rearrange("p (t s) -> p t s", t=CT)
                for h in range(NH):
                    pr, hi = h // 2, 64 * (h % 2)
                    for tk in range(CT):
                        nc.tensor.matmul(
                            out_ps[hi:hi + 64, pr, :],
                            wvt_s[:, tk, h * DH:(h + 1) * DH],
                            wt_sb[:, tk, h::32],
                            start=(tk == 0), stop=(tk == CT - 1),
                            tile_position=(0, hi),
                            skip_group_check=True)
                out_sb = wp.tile([128, CT, GRP], F32, tag="outsb")
                for pr in range(CT):
                    nc.vector.tensor_scalar_add(out_sb[:, pr, :],
                                                out_ps[:, pr, :],
                                                bv_s[:, pr:pr + 1])
                nc.sync.dma_start(out=out_d[:, g], in_=out_sb[:])

            # ---------------- interleaved schedule ----------------
            sc0 = phase_a(0)
            e0 = phase_b_exp(sc0)
            sc1 = phase_a(1)
            b0 = phase_b_tp(*e0)
            e1 = phase_b_exp(sc1)
            b1 = phase_b_tp(*e1)
            phase_c(0, b0[1], b0[2], b0[3])
            phase_c(1, b1[1], b1[2], b1[3])

    nc.compile()
    return nc


def _prep(pos_emb, Wq, bq, Wk, bk, Wv, bv):
    import ml_dtypes
    bf = ml_dtypes.bfloat16

    def ptn(v):  # [512] -> [128, CT], c = t*128 + p
        return np.ascontiguousarray(v.reshape(CT, 128).T)

    def chunkk(w):  # [512, N] -> [128, CT, N], k = t*128 + p
        return np.ascontiguousarray(w.reshape(CT, 128, -1).transpose(1, 0, 2))

    p1 = pos_emb[1:].sum(axis=0)
    pos0adj = (pos_emb[0] - p1 / HW).astype(np.float32)
    qcv = ISQ * (Wq.astype(np.float64) @ pos0adj.astype(np.float64)
                 + bq.astype(np.float64)).astype(np.float32)
    mask = np.zeros((128, CT, 32), np.float32)
    for p in range(128):
        for t in range(CT):
            h = (t * 128 + p) // DH
            mask[p, t, h] = 1.0

    return {
        "wqt": chunkk(np.ascontiguousarray(Wq.T)).astype(bf),
        "wk": chunkk(Wk).astype(bf),
        "wvt": chunkk(np.ascontiguousarray(Wv.T)).astype(bf),
        "pos0col": ptn(pos0adj).astype(bf),
        "pos0bc": np.ascontiguousarray(
            np.broadcast_to(pos0adj.reshape(1, C), (128, C))).astype(bf),
        "qc": ptn(qcv),
        "bv": ptn(bv),
        "mask32": mask.astype(bf),
        "ident": np.eye(128, dtype=np.float32).astype(bf),
        "ident32": np.tile(np.eye(32, dtype=np.float32), (4, 1)).astype(bf),
    }


def _prep_x(x, pos_emb):
    """(x + pos) [B, C, HW] -> per-core (xc, xt) bf16 DMA-friendly layouts."""
    import ml_dtypes
    bf = ml_dtypes.bfloat16
    xb = (x + pos_emb[1:].T[None]).astype(bf)
    xcs, xts = [], []
    for i in range(NCORES):
        xs = xb[i * BPC:(i + 1) * BPC]                       # [BPC, C, HW]
        xc = xs.reshape(BPC, CT, 128, HW).transpose(0, 2, 1, 3)
        xcs.append(np.ascontiguousarray(xc))                 # [BPC,128,CT,HW]
        xt = xs.transpose(0, 2, 1).reshape(BPC, JT, 128, C)
        xt = xt.transpose(0, 2, 1, 3)
        xts.append(np.ascontiguousarray(xt))                 # [BPC,128,JT,C]
    return xcs, xts


def _in_maps(x, pos_emb, Wq, bq, Wk, bk, Wv, bv):
    shared = _prep(np.asarray(pos_emb, np.float32), np.asarray(Wq, np.float32),
                   np.asarray(bq, np.float32), np.asarray(Wk, np.float32),
                   np.asarray(bk, np.float32), np.asarray(Wv, np.float32),
                   np.asarray(bv, np.float32))
    xcs, xts = _prep_x(np.asarray(x, np.float32).reshape(B, C, HW),
                       np.asarray(pos_emb, np.float32))
    in_maps = []
    for i in range(NCORES):
        m = dict(shared)
        m["xc"] = xcs[i]
        m["xt"] = xts[i]
        in_maps.append(m)
    return in_maps


def kernel(x, pos_emb, Wq, bq, Wk, bk, Wv, bv, num_heads):
    assert int(num_heads) == NH
    if "nc" not in _CACHE:
        _CACHE["nc"] = _build_nc()
    nc = _CACHE["nc"]
    in_maps = _in_maps(x, pos_emb, Wq, bq, Wk, bk, Wv, bv)
    res = run_bass_kernel_spmd(nc, in_maps, list(range(NCORES)))
    # out_d [128, NGRP, CT, GRP] -> [BPC, C]
    outs = []
    for i in range(NCORES):
        v = res.results[i]["out"]                 # [128, NGRP, CT, GRP]
        o = v.transpose(1, 3, 2, 0).reshape(BPC, C)   # [g, s, t, p] -> b, c
        outs.append(o)
    return np.concatenate(outs, axis=0).astype(np.float32)
